# revision 10
# baseline (speedup 1.0000x reference)
"""BitLinear (ternary-weight + int8-activation fake-quant linear) on 8 TRN2 cores.

Reference computation (all f32):
    gamma  = max(|x|) (global)          -> scale s = 127/gamma
    x_q    = round(x*s)/s               (RNE, no clip needed: |x*s| <= 127)
    gw     = mean(|W|) (global)
    w_q    = clip(round(W/gw), -1, 1) * gw
    out    = x_q @ w_q.T + b

Kernel strategy (data-parallel over rows of x, W replicated):
  - x_int = round(x*s) in [-127,127] and w_int in {-1,0,1} are integers that
    are exact in bf16; their <=2048-term dot products are exact in f32 PSUM.
    So the matmul runs in bf16 at full PE rate with *exact* integer results,
    and the output is rescaled once by c = gw/s.
  - Host prep: x is reshaped to (16384, 2048), row-sharded 8 ways, and each
    shard transposed to (2048_i, 2048_m) so the contraction dim lands on
    SBUF partitions; W is transposed once to W^T (2048_i, 2048_o).
  - Pass A (per core): absmax over the local x shard + sum|W| over a 1/8
    row-shard of W^T; one AllGather of the two partials; scalars
    s, 1/gw, c are derived on-device and broadcast via DMA.
  - Pass B: stream x/W^T, fake-quant to bf16 with the magic-number RNE trick
    (t = v*scale + 1.5*2^23; t -= 1.5*2^23), 16x16x4 tiled matmul into PSUM,
    epilogue rescale + bias add, stream out.
"""

import os
from contextlib import ExitStack

import numpy as np

import concourse.bass as bass
import concourse.mybir as mybir
import concourse.tile as tile
from concourse import bacc
from concourse.bass import ds, ts
from concourse.bass_utils import run_bass_kernel_spmd

F32 = mybir.dt.float32
BF16 = mybir.dt.bfloat16
AX = mybir.AxisListType
ALU = mybir.AluOpType
ACTF = mybir.ActivationFunctionType

MAGIC = 12582912.0  # 1.5 * 2**23: (v + MAGIC) - MAGIC == round-nearest-even(v)
Q_MAX = 127.0
EPS = 1e-8


def build_bass(I=2048, O=2048, MS=2048, cores=8):
    """Emit the per-core SPMD program. I: in_features, O: out_features,
    MS: rows of x per core. All must be multiples of 128 (O of 512)."""
    P = 128
    KT = I // P          # contraction tiles
    MT = MS // P         # output row tiles per core
    NO = 512
    OC = O // NO         # output col chunks
    WSH = I // cores     # rows of W^T this core reduces for sum|W|

    nc = bacc.Bacc(
        "TRN2",
        target_bir_lowering=False,
        debug=False,
        enable_asserts=True,
        num_devices=cores,
    )

    xt = nc.dram_tensor("xt", [I, MS], F32, kind="ExternalInput")
    wt = nc.dram_tensor("wt", [I, O], F32, kind="ExternalInput")
    wsh = nc.dram_tensor("wsh", [WSH, O], F32, kind="ExternalInput")
    bias = nc.dram_tensor("bias", [1, O], F32, kind="ExternalInput")
    out = nc.dram_tensor("out", [MS, O], F32, kind="ExternalOutput")

    with tile.TileContext(nc) as tc, ExitStack() as ctx:
        # [128, O]-f32 streaming tiles shared by pass-A x scan, the W-shard
        # scan and the pass-B W stream (same shape -> one pool)
        p_stream = ctx.enter_context(tc.tile_pool(name="stream", bufs=3))
        p_red = ctx.enter_context(tc.tile_pool(name="red", bufs=1))
        p_wq = ctx.enter_context(tc.tile_pool(name="wq", bufs=KT))
        p_tw = ctx.enter_context(tc.tile_pool(name="tw", bufs=2))
        p_x = ctx.enter_context(tc.tile_pool(name="x", bufs=3))
        p_tx = ctx.enter_context(tc.tile_pool(name="tx", bufs=2))
        p_xq = ctx.enter_context(tc.tile_pool(name="xq", bufs=2))
        p_epi = ctx.enter_context(tc.tile_pool(name="epi", bufs=4))
        p_b = ctx.enter_context(tc.tile_pool(name="bias", bufs=1))
        p_sc = ctx.enter_context(tc.tile_pool(name="sc", bufs=1))
        p_ps = ctx.enter_context(tc.tile_pool(name="ps", bufs=2, space="PSUM"))
        p_dram = ctx.enter_context(tc.tile_pool(name="dram", bufs=1, space="DRAM"))

        # ---------------- pass A: local absmax(x), local sum|W| -------------
        xpart = p_red.tile([P, KT], F32, tag="xpart")
        for r in range(KT):
            sA = p_stream.tile([P, MS], F32, tag="stream")
            nc.sync.dma_start(out=sA[:], in_=xt[ts(r, P), :])
            nc.vector.tensor_reduce(
                out=xpart[:, ds(r, 1)], in_=sA[:], axis=AX.X, op=ALU.max,
                apply_absolute_value=True,
            )
        xcol = p_red.tile([P, 1], F32, tag="xcol")
        nc.vector.tensor_reduce(out=xcol[:], in_=xpart[:], axis=AX.X, op=ALU.max)
        xrow = p_red.tile([1, P], F32, tag="xrow")
        nc.sync.dma_start(out=xrow[:], in_=xcol[:])
        graw = p_sc.tile([1, 1], F32, tag="graw")
        nc.vector.tensor_reduce(out=graw[:], in_=xrow[:], axis=AX.X, op=ALU.max)

        nwsh = (WSH + P - 1) // P
        wpart = p_red.tile([P, nwsh], F32, tag="wpart")
        if WSH % P:
            nc.vector.memset(wpart[:], 0.0)
        for r in range(nwsh):
            rows = min(P, WSH - r * P)
            sW = p_stream.tile([P, O], F32, tag="stream")
            nc.sync.dma_start(out=sW[:rows, :], in_=wsh[ds(r * P, rows), :])
            nc.vector.tensor_reduce(
                out=wpart[:rows, ds(r, 1)], in_=sW[:rows, :], axis=AX.X,
                op=ALU.add, apply_absolute_value=True,
            )
        wcol = p_red.tile([P, 1], F32, tag="wcol")
        nc.vector.tensor_reduce(out=wcol[:], in_=wpart[:], axis=AX.X, op=ALU.add)
        wrow = p_red.tile([1, P], F32, tag="wrow")
        nc.sync.dma_start(out=wrow[:], in_=wcol[:])
        wsum = p_sc.tile([1, 1], F32, tag="wsum")
        nc.vector.tensor_reduce(out=wsum[:], in_=wrow[:], axis=AX.X, op=ALU.add)

        # ---------------- tiny AllGather of [absmax, sum|W|] -----------------
        sc_in = p_sc.tile([1, 2], F32, tag="scin")
        nc.vector.tensor_copy(out=sc_in[:, ds(0, 1)], in_=graw[:])
        nc.vector.tensor_copy(out=sc_in[:, ds(1, 1)], in_=wsum[:])
        cc_in = p_dram.tile([1, 2], F32, tag="ccin")
        cc_out = p_dram.tile([cores, 2], F32, tag="ccout")
        nc.gpsimd.dma_start(out=cc_in[:], in_=sc_in[:])
        nc.gpsimd.collective_compute(
            "AllGather",
            ALU.bypass,
            replica_groups=[list(range(cores))],
            ins=[cc_in[:].opt()],
            outs=[cc_out[:].opt()],
        )
        cc_sb = p_sc.tile([1, cores, 2], F32, tag="ccsb")
        nc.gpsimd.dma_start(out=cc_sb[:], in_=cc_out[:])

        # ---------------- scalars: s = 127/gamma, rw = 1/gw, c = gw/s --------
        gam = p_sc.tile([1, 1], F32, tag="gam")
        nc.vector.tensor_reduce(out=gam[:], in_=cc_sb[:, :, 0], axis=AX.X, op=ALU.max)
        nc.vector.tensor_scalar_max(out=gam[:], in0=gam[:], scalar1=EPS)
        s_sc = p_sc.tile([1, 1], F32, tag="s")
        nc.vector.reciprocal(out=s_sc[:], in_=gam[:])
        nc.vector.tensor_scalar_mul(out=s_sc[:], in0=s_sc[:], scalar1=Q_MAX)

        gw = p_sc.tile([1, 1], F32, tag="gw")
        nc.vector.tensor_reduce(out=gw[:], in_=cc_sb[:, :, 1], axis=AX.X, op=ALU.add)
        nc.vector.tensor_scalar_mul(out=gw[:], in0=gw[:], scalar1=1.0 / (I * O))
        nc.vector.tensor_scalar_max(out=gw[:], in0=gw[:], scalar1=EPS)
        rw = p_sc.tile([1, 1], F32, tag="rw")
        nc.vector.reciprocal(out=rw[:], in_=gw[:])
        c_sc = p_sc.tile([1, 1], F32, tag="c")
        nc.vector.reciprocal(out=c_sc[:], in_=s_sc[:])
        nc.vector.tensor_tensor(out=c_sc[:], in0=c_sc[:], in1=gw[:], op=ALU.mult)

        sc3 = p_sc.tile([1, 3], F32, tag="sc3")
        nc.vector.tensor_copy(out=sc3[:, ds(0, 1)], in_=s_sc[:])
        nc.vector.tensor_copy(out=sc3[:, ds(1, 1)], in_=rw[:])
        nc.vector.tensor_copy(out=sc3[:, ds(2, 1)], in_=c_sc[:])
        sc3_dram = p_dram.tile([1, 3], F32, tag="sc3d")
        nc.sync.dma_start(out=sc3_dram[:], in_=sc3[:])
        bc3 = p_sc.tile([P, 3], F32, tag="bc3")
        nc.sync.dma_start(out=bc3[:], in_=sc3_dram[:].to_broadcast((P, 3)))
        s_b = bc3[:, ds(0, 1)]
        rw_b = bc3[:, ds(1, 1)]
        c_b = bc3[:, ds(2, 1)]

        # ---------------- bias broadcast ------------------------------------
        btile = p_b.tile([P, O], F32, tag="bias")
        nc.sync.dma_start(out=btile[:], in_=bias[:, :].to_broadcast((P, O)))

        # per-partition constant (-MAGIC) used as ACT bias for the un-shift
        negC = p_sc.tile([P, 1], F32, tag="negC")
        nc.gpsimd.memset(negC[:], -MAGIC)

        # ---------------- W^T quantization: wq = clip(RNE(W*rw), -1, 1) -----
        wq = []
        for k in range(KT):
            wf = p_stream.tile([P, O], F32, tag="stream")
            nc.sync.dma_start(out=wf[:], in_=wt[ts(k, P), :])
            t1 = p_tw.tile([P, O], F32, tag="tw")
            nc.vector.tensor_scalar(
                out=t1[:], in0=wf[:], scalar1=rw_b, scalar2=MAGIC,
                op0=ALU.mult, op1=ALU.add,
            )
            nc.gpsimd.tensor_scalar(
                out=t1[:], in0=t1[:], scalar1=MAGIC - 1.0, scalar2=MAGIC + 1.0,
                op0=ALU.max, op1=ALU.min,
            )
            wqk = p_wq.tile([P, O], BF16, tag="wq")
            nc.scalar.activation(out=wqk[:], in_=t1[:], func=ACTF.Identity, bias=negC[:])
            wq.append(wqk)

        # ---------------- pass B: quantize x, matmul, epilogue --------------
        xt3 = xt[:, :].rearrange("(k p) m -> p k m", p=P)  # [128, KT, MS]
        for mi in range(MT):
            xm = p_x.tile([P, KT, P], F32, tag="x")
            nc.sync.dma_start(out=xm[:], in_=xt3[:, :, ts(mi, P)])
            tx = p_tx.tile([P, KT, P], F32, tag="tx")
            nc.vector.tensor_scalar(
                out=tx[:], in0=xm[:], scalar1=s_b, scalar2=MAGIC,
                op0=ALU.mult, op1=ALU.add,
            )
            xq = p_xq.tile([P, KT, P], BF16, tag="xq")
            nc.scalar.activation(out=xq[:], in_=tx[:], func=ACTF.Identity, bias=negC[:])

            pss = [
                p_ps.tile([P, NO], F32, tag=f"ps{o}", name=f"ps_{mi}_{o}")
                for o in range(OC)
            ]
            for k in range(KT):
                for o in range(OC):
                    nc.tensor.matmul(
                        pss[o][:],
                        lhsT=xq[:, k, :],
                        rhs=wq[k][:, ts(o, NO)],
                        start=(k == 0),
                        stop=(k == KT - 1),
                    )
            for o in range(OC):
                e = p_epi.tile([P, NO], F32, tag="e")
                nc.scalar.activation(out=e[:], in_=pss[o][:], func=ACTF.Copy, scale=c_b)
                eb = p_epi.tile([P, NO], F32, tag="eb")
                nc.vector.tensor_tensor(out=eb[:], in0=e[:], in1=btile[:, ts(o, NO)], op=ALU.add)
                nc.sync.dma_start(out=out[ts(mi, P), ts(o, NO)], in_=eb[:])

    nc.compile()
    return nc


_NC_CACHE = {}
TRACE = False
LAST_RESULTS = None


def _get_nc(key, **kw):
    if key not in _NC_CACHE:
        _NC_CACHE[key] = build_bass(**kw)
    return _NC_CACHE[key]


def kernel(x: np.ndarray, W: np.ndarray, b: np.ndarray) -> np.ndarray:
    global LAST_RESULTS
    CORES = 8
    B, S, I = x.shape
    O = W.shape[0]
    R = B * S
    MS = R // CORES

    nc = _get_nc((I, O, MS, CORES), I=I, O=O, MS=MS, cores=CORES)

    xf = np.ascontiguousarray(x, dtype=np.float32).reshape(R, I)
    WT = np.ascontiguousarray(W.T.astype(np.float32, copy=False))  # [I, O]
    b2 = np.ascontiguousarray(b.astype(np.float32, copy=False)).reshape(1, O)
    WSH = I // CORES

    in_maps = []
    for c in range(CORES):
        xts = np.ascontiguousarray(xf[c * MS:(c + 1) * MS, :].T)  # [I, MS]
        in_maps.append({
            "xt": xts,
            "wt": WT,
            "wsh": np.ascontiguousarray(WT[c * WSH:(c + 1) * WSH, :]),
            "bias": b2,
        })

    res = run_bass_kernel_spmd(
        nc, in_maps, core_ids=list(range(CORES)), trace=TRACE,
    )
    LAST_RESULTS = res
    outs = [res.results[c]["out"] for c in range(CORES)]
    return np.concatenate(outs, axis=0).reshape(B, S, O).astype(np.float32)


# revision 11
# speedup vs baseline: 1.7673x; 1.7673x over previous
"""BitLinear (ternary-weight + int8-activation fake-quant linear) on 8 TRN2 cores.

Reference computation (all f32):
    gamma  = max(|x|) (global)          -> scale s = 127/gamma
    x_q    = round(x*s)/s               (RNE, no clip needed: |x*s| <= 127)
    gw     = mean(|W|) (global)
    w_q    = clip(round(W/gw), -1, 1) * gw
    out    = x_q @ w_q.T + b

Kernel strategy (data-parallel over rows of x, W replicated):
  - x_int = round(x*s) in [-127,127] and w_int in {-1,0,1} are integers that
    are exact in bf16; their <=2048-term dot products are exact in f32 PSUM.
    So the matmul runs in bf16 at full PE rate with *exact* integer results,
    and the output is rescaled once by c = gw/s.
  - Host prep: x is reshaped to (16384, 2048), row-sharded 8 ways, and each
    shard transposed to (2048_i, 2048_m) so the contraction dim lands on
    SBUF partitions; W is transposed once to W^T (2048_i, 2048_o).
  - Pass A (per core): absmax over the local x shard + sum|W| over a 1/8
    row-shard of W^T, both kept per-partition [128,2]; one AllGather moves
    the 8 cores' columns; scalars s, 1/gw, c derived on-device + broadcast.
  - Quant: x and W^T stream in contiguous [128, 2048] k-chunks, are
    magic-rounded in-place on the Scalar engine (t = v*scale + 1.5*2^23
    rounds to nearest-even in f32), clamped (W only) on DVE, and written as
    resident bf16 tiles xq[k] / wq[k].
  - Matmul: for each of 16 output row tiles, 16 k-accumulations x 4 output
    chunks of 512 into PSUM; epilogue rescales by c (ACT) and adds bias
    (DVE) then streams out.
"""

from contextlib import ExitStack

import numpy as np

import concourse.bass as bass
import concourse.mybir as mybir
import concourse.tile as tile
from concourse import bacc
from concourse.bass import ds, ts
from concourse.bass_utils import run_bass_kernel_spmd

F32 = mybir.dt.float32
BF16 = mybir.dt.bfloat16
AX = mybir.AxisListType
ALU = mybir.AluOpType
ACTF = mybir.ActivationFunctionType

MAGIC = 12582912.0  # 1.5 * 2**23: (v + MAGIC) - MAGIC == round-nearest-even(v)
Q_MAX = 127.0
EPS = 1e-8


def build_bass(I=2048, O=2048, MS=2048, cores=8):
    """Emit the per-core SPMD program. I: in_features, O: out_features,
    MS: rows of x per core. All must be multiples of 128 (O of 512)."""
    P = 128
    KT = I // P          # contraction tiles
    MT = MS // P         # output row tiles per core
    NO = 512
    OC = O // NO         # output col chunks
    WSH = I // cores     # rows of W^T this core reduces for sum|W|

    nc = bacc.Bacc(
        "TRN2",
        target_bir_lowering=False,
        debug=False,
        enable_asserts=True,
        num_devices=cores,
    )

    xt = nc.dram_tensor("xt", [I, MS], F32, kind="ExternalInput")
    wt = nc.dram_tensor("wt", [I, O], F32, kind="ExternalInput")
    wsh = nc.dram_tensor("wsh", [WSH, O], F32, kind="ExternalInput")
    bias = nc.dram_tensor("bias", [1, O], F32, kind="ExternalInput")
    out = nc.dram_tensor("out", [MS, O], F32, kind="ExternalOutput")

    with tile.TileContext(nc) as tc, ExitStack() as ctx:
        p_xs = ctx.enter_context(tc.tile_pool(name="xs", bufs=2))    # x stream f32
        p_ws = ctx.enter_context(tc.tile_pool(name="ws", bufs=2))    # W stream f32
        p_red = ctx.enter_context(tc.tile_pool(name="red", bufs=1))
        p_wq = ctx.enter_context(tc.tile_pool(name="wq", bufs=KT))   # resident bf16
        p_xq = ctx.enter_context(tc.tile_pool(name="xq", bufs=KT))   # resident bf16
        p_e = ctx.enter_context(tc.tile_pool(name="e", bufs=2))
        p_eb = ctx.enter_context(tc.tile_pool(name="eb", bufs=3))
        p_b = ctx.enter_context(tc.tile_pool(name="bias", bufs=1))
        p_sc = ctx.enter_context(tc.tile_pool(name="sc", bufs=1))
        p_ps = ctx.enter_context(tc.tile_pool(name="ps", bufs=2, space="PSUM"))
        p_dram = ctx.enter_context(tc.tile_pool(name="dram", bufs=1, space="DRAM"))

        # ---------------- pass A: local absmax(x), local sum|W| -------------
        xpart = p_red.tile([P, KT], F32, tag="xpart")
        for r in range(KT):
            sA = p_xs.tile([P, MS], F32, tag="xs", name=f"scan{r}")
            nc.sync.dma_start(out=sA[:], in_=xt[ts(r, P), :])
            nc.vector.tensor_reduce(
                out=xpart[:, ds(r, 1)], in_=sA[:], axis=AX.X, op=ALU.max,
                apply_absolute_value=True,
            )

        nwsh = (WSH + P - 1) // P
        wpart = p_red.tile([P, nwsh], F32, tag="wpart")
        if WSH % P:
            nc.vector.memset(wpart[:], 0.0)
        for r in range(nwsh):
            rows = min(P, WSH - r * P)
            sW = p_ws.tile([P, O], F32, tag="ws", name=f"wscan{r}")
            nc.sync.dma_start(out=sW[:rows, :], in_=wsh[ds(r * P, rows), :])
            nc.vector.tensor_reduce(
                out=wpart[:rows, ds(r, 1)], in_=sW[:rows, :], axis=AX.X,
                op=ALU.add, apply_absolute_value=True,
            )

        # per-partition payload [absmax | wsum] -> AllGather across 8 cores
        pay = p_red.tile([P, 2], F32, tag="pay")
        nc.vector.tensor_reduce(out=pay[:, ds(0, 1)], in_=xpart[:], axis=AX.X, op=ALU.max)
        nc.vector.tensor_reduce(out=pay[:, ds(1, 1)], in_=wpart[:], axis=AX.X, op=ALU.add)
        cc_in = p_dram.tile([P, 2], F32, tag="ccin")
        cc_out = p_dram.tile([cores * P, 2], F32, tag="ccout")
        nc.sync.dma_start(out=cc_in[:], in_=pay[:])
        nc.gpsimd.collective_compute(
            "AllGather",
            ALU.bypass,
            replica_groups=[list(range(cores))],
            ins=[cc_in[:].opt()],
            outs=[cc_out[:].opt()],
        )
        gath = p_sc.tile([1, cores * P, 2], F32, tag="gath")
        nc.sync.dma_start(out=gath[:], in_=cc_out[:])

        # ---------------- scalars: s = 127/gamma, rw = 1/gw, c = gw/s --------
        gam = p_sc.tile([1, 1], F32, tag="gam")
        nc.vector.tensor_reduce(out=gam[:], in_=gath[:, :, 0], axis=AX.X, op=ALU.max)
        nc.vector.tensor_scalar_max(out=gam[:], in0=gam[:], scalar1=EPS)
        s_sc = p_sc.tile([1, 1], F32, tag="s")
        nc.vector.reciprocal(out=s_sc[:], in_=gam[:])
        nc.vector.tensor_scalar_mul(out=s_sc[:], in0=s_sc[:], scalar1=Q_MAX)

        gw = p_sc.tile([1, 1], F32, tag="gw")
        nc.vector.tensor_reduce(out=gw[:], in_=gath[:, :, 1], axis=AX.X, op=ALU.add)
        nc.vector.tensor_scalar_mul(out=gw[:], in0=gw[:], scalar1=1.0 / (I * O))
        nc.vector.tensor_scalar_max(out=gw[:], in0=gw[:], scalar1=EPS)
        rw = p_sc.tile([1, 1], F32, tag="rw")
        nc.vector.reciprocal(out=rw[:], in_=gw[:])
        c_sc = p_sc.tile([1, 1], F32, tag="c")
        nc.vector.reciprocal(out=c_sc[:], in_=s_sc[:])
        nc.vector.tensor_tensor(out=c_sc[:], in0=c_sc[:], in1=gw[:], op=ALU.mult)

        sc3 = p_sc.tile([1, 3], F32, tag="sc3")
        nc.vector.tensor_copy(out=sc3[:, ds(0, 1)], in_=s_sc[:])
        nc.vector.tensor_copy(out=sc3[:, ds(1, 1)], in_=rw[:])
        nc.vector.tensor_copy(out=sc3[:, ds(2, 1)], in_=c_sc[:])
        sc3_dram = p_dram.tile([1, 3], F32, tag="sc3d")
        nc.sync.dma_start(out=sc3_dram[:], in_=sc3[:])
        bc3 = p_sc.tile([P, 3], F32, tag="bc3")
        nc.sync.dma_start(out=bc3[:], in_=sc3_dram[:].to_broadcast((P, 3)))
        s_b = bc3[:, ds(0, 1)]
        rw_b = bc3[:, ds(1, 1)]
        c_b = bc3[:, ds(2, 1)]

        # ---------------- bias broadcast + constants ------------------------
        btile = p_b.tile([P, O], F32, tag="bias")
        nc.sync.dma_start(out=btile[:], in_=bias[:, :].to_broadcast((P, O)))
        negC = p_sc.tile([P, 1], F32, tag="negC")
        nc.gpsimd.memset(negC[:], -MAGIC)

        # ---------------- W^T quant: wq = clip(RNE(W*rw), -1, 1) ------------
        # in-place magic round on the stream tile, then downconvert to bf16
        wq = []
        for k in range(KT):
            wf = p_ws.tile([P, O], F32, tag="ws", name=f"wf{k}")
            nc.sync.dma_start(out=wf[:], in_=wt[ts(k, P), :])
            nc.scalar.activation(out=wf[:], in_=wf[:], func=ACTF.Copy,
                                 scale=rw_b, bias=MAGIC)
            nc.vector.tensor_scalar(
                out=wf[:], in0=wf[:], scalar1=MAGIC - 1.0, scalar2=MAGIC + 1.0,
                op0=ALU.max, op1=ALU.min,
            )
            wqk = p_wq.tile([P, O], BF16, tag="wq", name=f"wq{k}")
            nc.scalar.activation(out=wqk[:], in_=wf[:], func=ACTF.Identity, bias=negC[:])
            wq.append(wqk)

        # ---------------- x quant: xq = RNE(x*s) ----------------------------
        xq = []
        for k in range(KT):
            xf = p_xs.tile([P, MS], F32, tag="xs", name=f"xf{k}")
            nc.sync.dma_start(out=xf[:], in_=xt[ts(k, P), :])
            nc.scalar.activation(out=xf[:], in_=xf[:], func=ACTF.Copy,
                                 scale=s_b, bias=MAGIC)
            xqk = p_xq.tile([P, MS], BF16, tag="xq", name=f"xq{k}")
            nc.scalar.activation(out=xqk[:], in_=xf[:], func=ACTF.Identity, bias=negC[:])
            xq.append(xqk)

        # ---------------- matmul + epilogue ---------------------------------
        for mi in range(MT):
            pss = [
                p_ps.tile([P, NO], F32, tag=f"ps{o}", name=f"ps_{mi}_{o}")
                for o in range(OC)
            ]
            for k in range(KT):
                lhsT = xq[k][:, ts(mi, P)]
                for o in range(OC):
                    nc.tensor.matmul(
                        pss[o][:],
                        lhsT=lhsT,
                        rhs=wq[k][:, ts(o, NO)],
                        start=(k == 0),
                        stop=(k == KT - 1),
                    )
            for o in range(OC):
                e = p_e.tile([P, NO], F32, tag="e", name=f"e_{mi}_{o}")
                nc.scalar.activation(out=e[:], in_=pss[o][:], func=ACTF.Copy, scale=c_b)
                eb = p_eb.tile([P, NO], F32, tag="eb", name=f"eb_{mi}_{o}")
                nc.vector.tensor_tensor(out=eb[:], in0=e[:], in1=btile[:, ts(o, NO)], op=ALU.add)
                nc.sync.dma_start(out=out[ts(mi, P), ts(o, NO)], in_=eb[:])

    nc.compile()
    return nc


_NC_CACHE = {}
TRACE = False
LAST_RESULTS = None


def _get_nc(key, **kw):
    if key not in _NC_CACHE:
        _NC_CACHE[key] = build_bass(**kw)
    return _NC_CACHE[key]


def kernel(x: np.ndarray, W: np.ndarray, b: np.ndarray) -> np.ndarray:
    global LAST_RESULTS
    CORES = 8
    B, S, I = x.shape
    O = W.shape[0]
    R = B * S
    MS = R // CORES

    nc = _get_nc((I, O, MS, CORES), I=I, O=O, MS=MS, cores=CORES)

    xf = np.ascontiguousarray(x, dtype=np.float32).reshape(R, I)
    WT = np.ascontiguousarray(W.T.astype(np.float32, copy=False))  # [I, O]
    b2 = np.ascontiguousarray(b.astype(np.float32, copy=False)).reshape(1, O)
    WSH = I // CORES

    in_maps = []
    for c in range(CORES):
        xts = np.ascontiguousarray(xf[c * MS:(c + 1) * MS, :].T)  # [I, MS]
        in_maps.append({
            "xt": xts,
            "wt": WT,
            "wsh": np.ascontiguousarray(WT[c * WSH:(c + 1) * WSH, :]),
            "bias": b2,
        })

    res = run_bass_kernel_spmd(
        nc, in_maps, core_ids=list(range(CORES)), trace=TRACE,
    )
    LAST_RESULTS = res
    outs = [res.results[c]["out"] for c in range(CORES)]
    return np.concatenate(outs, axis=0).reshape(B, S, O).astype(np.float32)
